# revision 31
# baseline (speedup 1.0000x reference)
"""Trainium2 Bass kernel: differentiable Gaussian-splat renderer.

Math: image[b,h,w,c] = clip( sum_n exp(-a_n*((gx_w-px_n)^2+(gy_h-py_n)^2)) * col[n,c], 0, 1 )
with a_n = 1/(2*sigma_n^2+1e-8), sigma_n = sizes_n*2/H.

The Gaussian separates: exp(-a*(dx^2+dy^2)) = exp(-a*dx^2)*exp(-a*dy^2), so per
frame the image is a matmul over splats:
    image[h, (w,c)] = sum_n wy[n,h] * (wx[n,w]*col[n,c])

a*d2 is produced by a tiny K-dim polynomial matmul on the PE:
    a*d2[n, g] = a_n*g^2 + (-2*a_n*p_n)*g + a_n*p_n^2
then ONE Exp activation per chunk (constant scale -1) gives wx|wy for both
frames (PSUM bank = frame).

fp32r (11-bit mantissa) would destroy the d2 cancellation for small sigma, so
both polynomial operands are split hi/lo into fp32r pairs (error-free products,
~2^-22 effective precision) -- K grows 6->18 which costs nothing on the PE.
All per-point coefficient prep (incl. the hi/lo split and the [K, point]
transposed layout) happens on HOST at input-packing time; the device rep is a
pure render pipeline: z-matmul -> exp -> T-build -> main matmul -> clip -> DMA.

T-build is one tensor_tensor per (frame, chunk): broadcast wx times a
once-per-call color tile replicated over w (frame 0 on DVE, frame 1 on
GpSimd). Clips: frame 0 on DVE as min(x,1); frame 1 on the scalar engine as
Relu(1-x) (host un-inverts) so DVE/Act split the PSUM-drain work. Each frame
stores with a single DMA (112+112 h-split, 2-segment DRAM AP). Reps are
software-pipelined: pass1(k) issues before pass2(k-1), keeping the PE stream
dense and keeping late-rep clips out of the next rep's engine queues.

NOTE: 16-bit (fp16/bf16) SBUF->DRAM output DMA silently corrupts every other
2-byte element on this HW (first 6 rows clean) -- outputs stay f32; a
gpsimd cast-DMA (SWDGE) is the clean alternative but costs ~1us/transfer of
Pool-engine descriptor generation.

Sharding: data-parallel over B: 16 frames -> 8 cores x 2 frames.
"""

import numpy as np

H = 224
W = 224
NPTS = 381
CH = 3
B = 16
NCORES = 8
BPC = B // NCORES    # frames per core
NCHUNK = 3           # point chunks; n = 3*p + j  (381 = 127*3)
KC = NPTS // NCHUNK  # 127 points per chunk (contraction partitions)
NH = 336             # moving-dim half (672 = 2*336); >=256 keeps f32r at full rate

HQ = 112             # h rows per output group (2 groups of 112 = one frame)
# frames whose clips run on the scalar engine as Relu(1-x) (host un-inverts)
ACT_B = {1}
# frames whose T-build runs on gpsimd (rest on vector)
POOL_B = {1}
PS_Z_BUFS = 2
PS_OUT_BUFS = 2
REPS = 1  # repeat whole body (benchmarking only)


def _round_f32r(x):
    """Round float32 array to fp32r (keep top 11 mantissa bits, round-nearest)."""
    u = np.ascontiguousarray(x, dtype=np.float32).view(np.uint32)
    low = u & 0xFFF
    up = (low > 0x800) | ((low == 0x800) & (((u >> 12) & 1) == 1))
    r = (u & ~np.uint32(0xFFF)) + np.where(up, np.uint32(0x1000), np.uint32(0))
    return r.view(np.float32)


def _np_grid96():
    """[96, 448] fp32r rows; chunk j occupies rows [32j, 32j+18).
    Within a chunk, row 3r+t pairs with lhs row t in {hi: R_hi, hi: R_lo, lo: R_hi}.
    r in 0..5 = (g^2, g, 1) for x-half cols [0:224], same for y-half [224:448]."""
    g = -1.0 + (2.0 / (W - 1)) * np.arange(W, dtype=np.float64)
    R = np.zeros((6, 2 * W), dtype=np.float64)
    R[0, 0:W] = g * g
    R[1, 0:W] = g
    R[2, 0:W] = 1.0
    R[3, W:] = g * g
    R[4, W:] = g
    R[5, W:] = 1.0
    g18 = np.zeros((18, 2 * W), dtype=np.float32)
    for r in range(6):
        hi = _round_f32r(R[r].astype(np.float32))
        lo = _round_f32r((R[r] - hi.astype(np.float64)).astype(np.float32))
        g18[3 * r + 0] = hi   # pairs L_hi
        g18[3 * r + 1] = lo   # pairs L_hi
        g18[3 * r + 2] = hi   # pairs L_lo
    out = np.zeros((96, 2 * W), dtype=np.float32)
    for j in range(NCHUNK):
        out[32 * j : 32 * j + 18] = g18
    return out


def _pack_lhs(positions, sizes):
    """[B, 96, 128] f32: lhsT coeff rows (a, -2ap, ap^2 for x then y, hi/lo
    split, row 32j+3r+t) by point column p; n = NCHUNK*p + j. Col 127 = 0."""
    p64 = positions.astype(np.float64)            # [B, N, 2]
    s64 = sizes.astype(np.float64)                # [B, N]
    sigma = s64 * (2.0 / H)
    a = 1.0 / (2.0 * sigma * sigma + 1e-8)        # [B, N]
    Lr = np.zeros((B, NPTS, 6), dtype=np.float64)
    Lr[:, :, 0] = a
    Lr[:, :, 1] = -2.0 * a * p64[:, :, 0]
    Lr[:, :, 2] = a * p64[:, :, 0] * p64[:, :, 0]
    Lr[:, :, 3] = a
    Lr[:, :, 4] = -2.0 * a * p64[:, :, 1]
    Lr[:, :, 5] = a * p64[:, :, 1] * p64[:, :, 1]
    hi = _round_f32r(Lr.astype(np.float32))
    lo = _round_f32r((Lr - hi.astype(np.float64)).astype(np.float32))
    hiR = hi.reshape(B, KC, NCHUNK, 6)            # [b, p, j, r]
    loR = lo.reshape(B, KC, NCHUNK, 6)
    lhs = np.zeros((B, 96, 128), dtype=np.float32)
    for j in range(NCHUNK):
        for r in range(6):
            lhs[:, 32 * j + 3 * r + 0, 0:KC] = hiR[:, :, j, r]
            lhs[:, 32 * j + 3 * r + 1, 0:KC] = hiR[:, :, j, r]
            lhs[:, 32 * j + 3 * r + 2, 0:KC] = loR[:, :, j, r]
    return lhs


def _pack_colors(colors):
    """[B, KC, NCHUNK, CH] f32, point-partition-major."""
    return np.ascontiguousarray(
        colors.astype(np.float32).reshape(B, KC, NCHUNK, CH)
    )


def build_bass():
    import concourse.bass as bass
    import concourse.bacc as bacc
    import concourse.tile as tile
    from concourse import mybir

    f32 = mybir.dt.float32
    f32r = mybir.dt.float32r
    f16 = mybir.dt.float16
    bf16 = mybir.dt.bfloat16
    Act = mybir.ActivationFunctionType
    Alu = mybir.AluOpType

    nc = bacc.Bacc("TRN2", debug=False, enable_partition_id=False)

    lhs_d = nc.dram_tensor("lhs", [96, BPC, 128], f32r, kind="ExternalInput")
    col_d = nc.dram_tensor("colors_pk", [KC, BPC, NCHUNK, CH], f32,
                           kind="ExternalInput")
    cst_d = nc.dram_tensor("consts", [96, 2 * W], f32r, kind="ExternalInput")
    img_d = nc.dram_tensor("image", [BPC, H, W, CH], f32, kind="ExternalOutput")

    with tile.TileContext(nc) as tc:
        with (
            tc.tile_pool(name="const", bufs=1) as constp,
            tc.tile_pool(name="big", bufs=2) as big,
            tc.tile_pool(name="outp", bufs=6) as outp,
            tc.tile_pool(name="ps_z", bufs=PS_Z_BUFS, space="PSUM") as ps_z,
            tc.tile_pool(name="ps_out", bufs=PS_OUT_BUFS, space="PSUM") as ps_out,
        ):
            grid96 = constp.tile([96, 2 * W], f32r)
            nc.scalar.dma_start(out=grid96, in_=cst_d[:])
            lhs_sb = constp.tile([96, BPC, 128], f32r)
            nc.sync.dma_start(out=lhs_sb, in_=lhs_d[:])
            col_sb = constp.tile([128, BPC, NCHUNK, CH], f32)
            nc.sync.dma_start(out=col_sb[0:KC], in_=col_d[:])

            # PE warmup: dummy matmuls so the HAM clock-gate opens before the
            # real pipeline reaches the PE (once; steady-state reps skip it)
            wsb = constp.tile([1, 128], f32)
            nc.vector.memset(wsb, 0.0)
            wps = ps_z.tile([128, BPC, 512], f32, tag="z")
            for _ in range(4):
                nc.tensor.matmul(wps[:, 0, 0:128], wsb, wsb, start=True, stop=True)

            def pass1(rep):
                """z matmuls (bank = frame), one exp per chunk, T-build
                spread over DVE/Pool. Returns (wxy, T_all)."""
                wxy = big.tile([128, NCHUNK, BPC, 2 * W], f32r, tag="wxy",
                               name=f"wxy_{rep}")
                T_all = big.tile([128, BPC, NCHUNK, CH * W], f32r, tag="T",
                                 name=f"T_{rep}")
                Twc = T_all.rearrange("p b j (w c) -> p b j w c", c=CH)
                for j in range(NCHUNK):
                    zp = ps_z.tile([128, BPC, 512], f32, tag="z",
                                   name=f"zp_{rep}_{j}")
                    for b in range(BPC):
                        nc.tensor.matmul(
                            zp[:, b, 0 : 2 * W],
                            lhs_sb[32 * j : 32 * j + 18, b, :],
                            grid96[32 * j : 32 * j + 18, :],
                            start=True, stop=True,
                        )
                    nc.scalar.activation(
                        out=wxy[0:KC, j], in_=zp[0:KC, :, 0 : 2 * W],
                        func=Act.Exp, scale=-1.0,
                    )
                    for b in range(BPC):
                        eng = nc.gpsimd if b in POOL_B else nc.vector
                        for c in range(CH):
                            eng.tensor_scalar(
                                out=Twc[0:KC, b, j, :, c],
                                in0=wxy[0:KC, j, b, 0:W],
                                scalar1=col_sb[0:KC, b, j, c : c + 1],
                                scalar2=None,
                                op0=Alu.mult,
                            )
                return wxy, T_all

            def pass2(rep, wxy, T_all):
                """main matmuls into 2-bank psum tiles (one per h-group),
                one strided clip per group (DVE min / Act Relu(1-x), host
                un-inverts Act frames), one whole-frame store per frame."""
                for b in range(BPC):
                    osb = outp.tile([128, 2 * W * CH], f32, tag="osb",
                                    name=f"osb_{b}_{rep}")
                    osf = osb.rearrange("p (q x s) -> p q x s", q=2, s=NH)
                    for q in range(2):
                        h0 = HQ * q
                        po = ps_out.tile([128, 2, 512], f32, tag="out",
                                         name=f"po_{b}_{q}_{rep}")
                        for j in range(NCHUNK):
                            for half in range(2):
                                nc.tensor.matmul(
                                    po[0:HQ, half, 0:NH],
                                    wxy[0:KC, j, b, W + h0 : W + h0 + HQ],
                                    T_all[0:KC, b, j, NH * half : NH * (half + 1)],
                                    start=(j == 0), stop=(j == NCHUNK - 1),
                                )
                        if b in ACT_B:
                            nc.scalar.activation(
                                out=osf[0:HQ, q], in_=po[0:HQ, :, 0:NH],
                                func=Act.Relu, scale=-1.0, bias=1.0,
                            )
                        else:
                            nc.vector.tensor_scalar(
                                out=osf[0:HQ, q], in0=po[0:HQ, :, 0:NH],
                                scalar1=1.0, scalar2=None, op0=Alu.min,
                            )
                    dma_eng = nc.scalar if b % 2 else nc.sync
                    dma_eng.dma_start(
                        out=img_d[b].rearrange("(q p) w c -> p q (w c)", q=2),
                        in_=osb[0:HQ].rearrange("p (q x) -> p q x", q=2),
                    )

            # software pipeline: pass1(k) issues before pass2(k-1), so the
            # PE main-matmul stream always consumes the PREVIOUS rep's
            # (complete) wxy/T and no engine queues early-rep work behind
            # late-rep clips
            prev = None
            for _rep in range(REPS):
                cur = pass1(_rep)
                if prev is not None:
                    pass2(_rep - 1, *prev)
                prev = cur
            pass2(REPS - 1, *prev)
    nc.compile()
    return nc


_CACHED = {}


def _get_bass():
    if "nc" not in _CACHED:
        _CACHED["nc"] = build_bass()
    return _CACHED["nc"]


def _post_tiles(imgs):
    """Device image tiles [..., BPC, H, W, CH] -> f32 with the Act-clipped
    frames (stored as 1-min(x,1)) un-inverted."""
    out = imgs.astype(np.float32)
    for b in ACT_B:
        out[..., b, :, :, :] = 1.0 - out[..., b, :, :, :]
    return out


def _postprocess(imgs):
    """[NCORES, BPC, H, W, CH] device tiles -> [B, H, W, CH] f32."""
    return _post_tiles(imgs).reshape(B, H, W, CH)


LAST_RESULT = None


def kernel(positions, colors, sizes, trace=False):
    from concourse.bass_utils import run_bass_kernel_spmd

    global LAST_RESULT
    positions = np.ascontiguousarray(np.asarray(positions, dtype=np.float32))
    colors = np.ascontiguousarray(np.asarray(colors, dtype=np.float32))
    sizes = np.ascontiguousarray(np.asarray(sizes, dtype=np.float32))

    lhs = _pack_lhs(positions, sizes)       # [B, 96, 128]
    col = _pack_colors(colors)              # [B, KC, NCHUNK, CH]
    cst = _np_grid96()
    nc = _get_bass()
    in_maps = []
    for c in range(NCORES):
        sl = slice(c * BPC, (c + 1) * BPC)
        in_maps.append({
            "lhs": np.ascontiguousarray(lhs[sl].transpose(1, 0, 2)),
            "colors_pk": np.ascontiguousarray(col[sl].transpose(1, 0, 2, 3)),
            "consts": cst,
        })

    res = run_bass_kernel_spmd(
        nc, in_maps, core_ids=list(range(NCORES)), trace=trace
    )
    LAST_RESULT = res
    imgs = np.stack([r["image"] for r in res.results], axis=0)
    return _postprocess(imgs)


def _exec_fn(nc):
    """Build a reusable jitted 8-core executor (no donation; kernel writes
    every output element so uninit result buffers are fine)."""
    import jax
    from jax.experimental.shard_map import shard_map
    from jax.sharding import Mesh, PartitionSpec
    from concourse import bass2jax, mybir

    bass2jax.install_neuronx_cc_hook()

    in_names, out_names, out_avals = [], [], []
    for alloc in nc.m.functions[0].allocations:
        if not isinstance(alloc, mybir.MemoryLocationSet):
            continue
        name = alloc.memorylocations[0].name
        if alloc.kind == "ExternalInput":
            in_names.append(name)
        elif alloc.kind == "ExternalOutput":
            out_names.append(name)
            out_avals.append(
                jax.core.ShapedArray(
                    tuple(alloc.tensor_shape), mybir.dt.np(alloc.dtype)
                )
            )
    all_in = in_names + out_names

    def _body(*args):
        outs = bass2jax._bass_exec_p.bind(
            *args,
            out_avals=tuple(out_avals),
            in_names=tuple(all_in),
            out_names=tuple(out_names),
            lowering_input_output_aliases=(),
            sim_require_finite=True,
            sim_require_nnan=True,
            nc=nc,
        )
        return tuple(outs)

    devices = jax.devices()[:NCORES]
    mesh = Mesh(np.asarray(devices), ("core",))
    n_args = len(all_in)
    sharded = jax.jit(
        shard_map(
            _body,
            mesh=mesh,
            in_specs=(PartitionSpec("core"),) * n_args,
            out_specs=(PartitionSpec("core"),) * len(out_names),
            check_rep=False,
        ),
        keep_unused=True,
    )
    return sharded, mesh, in_names, out_names, out_avals


def bench(positions, colors, sizes, iters=50):
    """Steady-state per-execution wall time (s) over 8 cores + output."""
    import time as _time
    import jax
    from jax.sharding import NamedSharding, PartitionSpec

    positions = np.ascontiguousarray(np.asarray(positions, dtype=np.float32))
    colors = np.ascontiguousarray(np.asarray(colors, dtype=np.float32))
    sizes = np.ascontiguousarray(np.asarray(sizes, dtype=np.float32))
    nc = _get_bass()
    sharded, mesh, in_names, out_names, out_avals = _exec_fn(nc)

    lhs = _pack_lhs(positions, sizes).reshape(NCORES, BPC, 96, 128)
    col = _pack_colors(colors).reshape(NCORES, BPC, KC, NCHUNK, CH)
    feed = {
        "lhs": np.ascontiguousarray(lhs.transpose(0, 2, 1, 3)).reshape(
            NCORES * 96, BPC, 128
        ),
        "colors_pk": np.ascontiguousarray(
            col.transpose(0, 2, 1, 3, 4)
        ).reshape(NCORES * KC, BPC, NCHUNK, CH),
        "consts": np.concatenate([_np_grid96()] * NCORES, axis=0),
    }
    args = [feed[n] for n in in_names]
    args += [
        np.zeros((NCORES * a.shape[0], *a.shape[1:]), a.dtype) for a in out_avals
    ]
    sh = NamedSharding(mesh, PartitionSpec("core"))
    dargs = [jax.device_put(a, sh) for a in args]

    out = sharded(*dargs)
    jax.block_until_ready(out)
    img0 = _postprocess(
        np.asarray(out[0]).reshape(NCORES, BPC, H, W, CH)
    )

    times = []
    for _ in range(3):
        t0 = _time.perf_counter()
        for _ in range(iters):
            out = sharded(*dargs)
        jax.block_until_ready(out)
        times.append((_time.perf_counter() - t0) / iters)
    return min(times), img0


# revision 32
# speedup vs baseline: 4.9402x; 4.9402x over previous
"""Trainium2 Bass kernel: differentiable Gaussian-splat renderer.

Math: image[b,h,w,c] = clip( sum_n exp(-a_n*((gx_w-px_n)^2+(gy_h-py_n)^2)) * col[n,c], 0, 1 )
with a_n = 1/(2*sigma_n^2+1e-8), sigma_n = sizes_n*2/H.

The Gaussian separates: exp(-a*(dx^2+dy^2)) = exp(-a*dx^2)*exp(-a*dy^2), so per
frame the image is a matmul over splats:
    image[h, (w,c)] = sum_n wy[n,h] * (wx[n,w]*col[n,c])

a*d2 is produced by a tiny K-dim polynomial matmul on the PE:
    a*d2[n, g] = a_n*g^2 + (-2*a_n*p_n)*g + a_n*p_n^2
then ONE Exp activation per chunk (constant scale -1) gives wx|wy for both
frames (PSUM bank = frame).

fp32r (11-bit mantissa) would destroy the d2 cancellation for small sigma, so
both polynomial operands are split hi/lo into fp32r pairs (error-free products,
~2^-22 effective precision) -- K grows 6->18 which costs nothing on the PE.
All per-point coefficient prep (incl. the hi/lo split and the [K, point]
transposed layout) happens on HOST at input-packing time; the device rep is a
pure render pipeline: z-matmul -> exp -> T-build -> main matmul -> clip -> DMA.

T-build is one tensor_tensor per (frame, chunk): broadcast wx times a
once-per-call color tile replicated over w (frame 0 on DVE, frame 1 on
GpSimd). Clips: frame 0 on DVE as min(x,1); frame 1 on the scalar engine as
Relu(1-x) (host un-inverts) so DVE/Act split the PSUM-drain work. Each frame
stores with a single DMA (112+112 h-split, 2-segment DRAM AP). Reps are
software-pipelined: pass1(k) issues before pass2(k-1), keeping the PE stream
dense and keeping late-rep clips out of the next rep's engine queues.

NOTE: 16-bit (fp16/bf16) SBUF->DRAM output DMA silently corrupts every other
2-byte element on this HW (first 6 rows clean) -- outputs stay f32; a
gpsimd cast-DMA (SWDGE) is the clean alternative but costs ~1us/transfer of
Pool-engine descriptor generation.

Sharding: data-parallel over B: 16 frames -> 8 cores x 2 frames.
"""

import numpy as np

H = 224
W = 224
NPTS = 381
CH = 3
B = 16
NCORES = 8
BPC = B // NCORES    # frames per core
NCHUNK = 3           # point chunks; n = 3*p + j  (381 = 127*3)
KC = NPTS // NCHUNK  # 127 points per chunk (contraction partitions)
NH = 336             # moving-dim half (672 = 2*336); >=256 keeps f32r at full rate

HQ = 112             # h rows per output group (2 groups of 112 = one frame)
# frames whose clips run on the scalar engine as Relu(1-x) (host un-inverts)
ACT_B = {1}
# frames whose T-build runs on gpsimd (rest on vector)
POOL_B = {1}
PS_Z_BUFS = 2
PS_OUT_BUFS = 2
REPS = 1  # repeat whole body (benchmarking only)


def _round_f32r(x):
    """Round float32 array to fp32r (keep top 11 mantissa bits, round-nearest)."""
    u = np.ascontiguousarray(x, dtype=np.float32).view(np.uint32)
    low = u & 0xFFF
    up = (low > 0x800) | ((low == 0x800) & (((u >> 12) & 1) == 1))
    r = (u & ~np.uint32(0xFFF)) + np.where(up, np.uint32(0x1000), np.uint32(0))
    return r.view(np.float32)


def _np_grid96():
    """[96, 448] fp32r rows; chunk j occupies rows [32j, 32j+18).
    Within a chunk, row 3r+t pairs with lhs row t in {hi: R_hi, hi: R_lo, lo: R_hi}.
    r in 0..5 = (g^2, g, 1) for x-half cols [0:224], same for y-half [224:448]."""
    g = -1.0 + (2.0 / (W - 1)) * np.arange(W, dtype=np.float64)
    R = np.zeros((6, 2 * W), dtype=np.float64)
    R[0, 0:W] = g * g
    R[1, 0:W] = g
    R[2, 0:W] = 1.0
    R[3, W:] = g * g
    R[4, W:] = g
    R[5, W:] = 1.0
    g18 = np.zeros((18, 2 * W), dtype=np.float32)
    for r in range(6):
        hi = _round_f32r(R[r].astype(np.float32))
        lo = _round_f32r((R[r] - hi.astype(np.float64)).astype(np.float32))
        g18[3 * r + 0] = hi   # pairs L_hi
        g18[3 * r + 1] = lo   # pairs L_hi
        g18[3 * r + 2] = hi   # pairs L_lo
    out = np.zeros((96, 2 * W), dtype=np.float32)
    for j in range(NCHUNK):
        out[32 * j : 32 * j + 18] = g18
    return out


def _pack_lhs(positions, sizes):
    """[B, 96, 128] f32: lhsT coeff rows (a, -2ap, ap^2 for x then y, hi/lo
    split, row 32j+3r+t) by point column p; n = NCHUNK*p + j. Col 127 = 0."""
    p64 = positions.astype(np.float64)            # [B, N, 2]
    s64 = sizes.astype(np.float64)                # [B, N]
    sigma = s64 * (2.0 / H)
    a = 1.0 / (2.0 * sigma * sigma + 1e-8)        # [B, N]
    Lr = np.zeros((B, NPTS, 6), dtype=np.float64)
    Lr[:, :, 0] = a
    Lr[:, :, 1] = -2.0 * a * p64[:, :, 0]
    Lr[:, :, 2] = a * p64[:, :, 0] * p64[:, :, 0]
    Lr[:, :, 3] = a
    Lr[:, :, 4] = -2.0 * a * p64[:, :, 1]
    Lr[:, :, 5] = a * p64[:, :, 1] * p64[:, :, 1]
    hi = _round_f32r(Lr.astype(np.float32))
    lo = _round_f32r((Lr - hi.astype(np.float64)).astype(np.float32))
    hiR = hi.reshape(B, KC, NCHUNK, 6)            # [b, p, j, r]
    loR = lo.reshape(B, KC, NCHUNK, 6)
    lhs = np.zeros((B, 96, 128), dtype=np.float32)
    for j in range(NCHUNK):
        for r in range(6):
            lhs[:, 32 * j + 3 * r + 0, 0:KC] = hiR[:, :, j, r]
            lhs[:, 32 * j + 3 * r + 1, 0:KC] = hiR[:, :, j, r]
            lhs[:, 32 * j + 3 * r + 2, 0:KC] = loR[:, :, j, r]
    return lhs


def _pack_colors(colors):
    """[B, KC, NCHUNK, CH] f32, point-partition-major."""
    return np.ascontiguousarray(
        colors.astype(np.float32).reshape(B, KC, NCHUNK, CH)
    )


def build_bass():
    import concourse.bass as bass
    import concourse.bacc as bacc
    import concourse.tile as tile
    from concourse import mybir

    f32 = mybir.dt.float32
    f32r = mybir.dt.float32r
    f16 = mybir.dt.float16
    bf16 = mybir.dt.bfloat16
    Act = mybir.ActivationFunctionType
    Alu = mybir.AluOpType

    nc = bacc.Bacc("TRN2", debug=False, enable_partition_id=False)

    lhs_d = nc.dram_tensor("lhs", [96, BPC, 128], f32r, kind="ExternalInput")
    col_d = nc.dram_tensor("colors_pk", [KC, BPC, NCHUNK, CH], f32,
                           kind="ExternalInput")
    cst_d = nc.dram_tensor("consts", [96, 2 * W], f32r, kind="ExternalInput")
    img_d = nc.dram_tensor("image", [BPC, H, W, CH], f32, kind="ExternalOutput")

    with tile.TileContext(nc) as tc:
        with (
            tc.tile_pool(name="const", bufs=1) as constp,
            tc.tile_pool(name="big", bufs=2) as big,
            tc.tile_pool(name="outp", bufs=6) as outp,
            tc.tile_pool(name="ps_z", bufs=PS_Z_BUFS, space="PSUM") as ps_z,
            tc.tile_pool(name="ps_out", bufs=PS_OUT_BUFS, space="PSUM") as ps_out,
        ):
            grid96 = constp.tile([96, 2 * W], f32r)
            nc.scalar.dma_start(out=grid96, in_=cst_d[:])
            lhs_sb = constp.tile([96, BPC, 128], f32r)
            nc.sync.dma_start(out=lhs_sb, in_=lhs_d[:])
            col_sb = constp.tile([128, BPC, NCHUNK, CH], f32)
            nc.sync.dma_start(out=col_sb[0:KC], in_=col_d[:])
            # one-time broadcast of colors over w so T-build is a single
            # tensor_tensor per (frame, chunk)
            col672 = constp.tile([128, BPC, NCHUNK, W, CH], f32)
            nc.vector.tensor_copy(
                out=col672[0:KC],
                in_=col_sb[0:KC].unsqueeze(3).broadcast_to(
                    [KC, BPC, NCHUNK, W, CH]
                ),
            )

            # PE warmup: dummy matmuls so the HAM clock-gate opens before the
            # real pipeline reaches the PE (once; steady-state reps skip it)
            wsb = constp.tile([1, 128], f32)
            nc.vector.memset(wsb, 0.0)
            wps = ps_z.tile([128, BPC, 512], f32, tag="z")
            for _ in range(4):
                nc.tensor.matmul(wps[:, 0, 0:128], wsb, wsb, start=True, stop=True)

            def pass1(rep):
                """z matmuls (bank = frame), one exp per chunk, T-build
                spread over DVE/Pool. Returns (wxy, T_all)."""
                wxy = big.tile([128, NCHUNK, BPC, 2 * W], f32r, tag="wxy",
                               name=f"wxy_{rep}")
                T_all = big.tile([128, BPC, NCHUNK, CH * W], f32r, tag="T",
                                 name=f"T_{rep}")
                Twc = T_all.rearrange("p b j (w c) -> p b j w c", c=CH)
                for j in range(NCHUNK):
                    zp = ps_z.tile([128, BPC, 512], f32, tag="z",
                                   name=f"zp_{rep}_{j}")
                    for b in range(BPC):
                        nc.tensor.matmul(
                            zp[:, b, 0 : 2 * W],
                            lhs_sb[32 * j : 32 * j + 18, b, :],
                            grid96[32 * j : 32 * j + 18, :],
                            start=True, stop=True,
                        )
                    nc.scalar.activation(
                        out=wxy[0:KC, j], in_=zp[0:KC, :, 0 : 2 * W],
                        func=Act.Exp, scale=-1.0,
                    )
                    for b in range(BPC):
                        eng = nc.gpsimd if b in POOL_B else nc.vector
                        eng.tensor_tensor(
                            out=Twc[0:KC, b, j],
                            in0=wxy[0:KC, j, b, 0:W].unsqueeze(2).broadcast_to(
                                [KC, W, CH]
                            ),
                            in1=col672[0:KC, b, j],
                            op=Alu.mult,
                        )
                return wxy, T_all

            def pass2(rep, wxy, T_all):
                """main matmuls into 2-bank psum tiles (one per h-group),
                one strided clip per group (DVE min / Act Relu(1-x), host
                un-inverts Act frames), one whole-frame store per frame."""
                for b in range(BPC):
                    osb = outp.tile([128, 2 * W * CH], f32, tag="osb",
                                    name=f"osb_{b}_{rep}")
                    osf = osb.rearrange("p (q x s) -> p q x s", q=2, s=NH)
                    for q in range(2):
                        h0 = HQ * q
                        po = ps_out.tile([128, 2, 512], f32, tag="out",
                                         name=f"po_{b}_{q}_{rep}")
                        for j in range(NCHUNK):
                            for half in range(2):
                                nc.tensor.matmul(
                                    po[0:HQ, half, 0:NH],
                                    wxy[0:KC, j, b, W + h0 : W + h0 + HQ],
                                    T_all[0:KC, b, j, NH * half : NH * (half + 1)],
                                    start=(j == 0), stop=(j == NCHUNK - 1),
                                )
                        if b in ACT_B:
                            nc.scalar.activation(
                                out=osf[0:HQ, q], in_=po[0:HQ, :, 0:NH],
                                func=Act.Relu, scale=-1.0, bias=1.0,
                            )
                        else:
                            nc.vector.tensor_scalar(
                                out=osf[0:HQ, q], in0=po[0:HQ, :, 0:NH],
                                scalar1=1.0, scalar2=None, op0=Alu.min,
                            )
                    dma_eng = nc.scalar if b % 2 else nc.sync
                    dma_eng.dma_start(
                        out=img_d[b].rearrange("(q p) w c -> p q (w c)", q=2),
                        in_=osb[0:HQ].rearrange("p (q x) -> p q x", q=2),
                    )

            # software pipeline: pass1(k) issues before pass2(k-1), so the
            # PE main-matmul stream always consumes the PREVIOUS rep's
            # (complete) wxy/T and no engine queues early-rep work behind
            # late-rep clips
            prev = None
            for _rep in range(REPS):
                cur = pass1(_rep)
                if prev is not None:
                    pass2(_rep - 1, *prev)
                prev = cur
            pass2(REPS - 1, *prev)
    nc.compile()
    return nc


_CACHED = {}


def _get_bass():
    if "nc" not in _CACHED:
        _CACHED["nc"] = build_bass()
    return _CACHED["nc"]


def _post_tiles(imgs):
    """Device image tiles [..., BPC, H, W, CH] -> f32 with the Act-clipped
    frames (stored as 1-min(x,1)) un-inverted."""
    out = imgs.astype(np.float32)
    for b in ACT_B:
        out[..., b, :, :, :] = 1.0 - out[..., b, :, :, :]
    return out


def _postprocess(imgs):
    """[NCORES, BPC, H, W, CH] device tiles -> [B, H, W, CH] f32."""
    return _post_tiles(imgs).reshape(B, H, W, CH)


LAST_RESULT = None


def kernel(positions, colors, sizes, trace=False):
    from concourse.bass_utils import run_bass_kernel_spmd

    global LAST_RESULT
    positions = np.ascontiguousarray(np.asarray(positions, dtype=np.float32))
    colors = np.ascontiguousarray(np.asarray(colors, dtype=np.float32))
    sizes = np.ascontiguousarray(np.asarray(sizes, dtype=np.float32))

    lhs = _pack_lhs(positions, sizes)       # [B, 96, 128]
    col = _pack_colors(colors)              # [B, KC, NCHUNK, CH]
    cst = _np_grid96()
    nc = _get_bass()
    in_maps = []
    for c in range(NCORES):
        sl = slice(c * BPC, (c + 1) * BPC)
        in_maps.append({
            "lhs": np.ascontiguousarray(lhs[sl].transpose(1, 0, 2)),
            "colors_pk": np.ascontiguousarray(col[sl].transpose(1, 0, 2, 3)),
            "consts": cst,
        })

    res = run_bass_kernel_spmd(
        nc, in_maps, core_ids=list(range(NCORES)), trace=trace
    )
    LAST_RESULT = res
    imgs = np.stack([r["image"] for r in res.results], axis=0)
    return _postprocess(imgs)


def _exec_fn(nc):
    """Build a reusable jitted 8-core executor (no donation; kernel writes
    every output element so uninit result buffers are fine)."""
    import jax
    from jax.experimental.shard_map import shard_map
    from jax.sharding import Mesh, PartitionSpec
    from concourse import bass2jax, mybir

    bass2jax.install_neuronx_cc_hook()

    in_names, out_names, out_avals = [], [], []
    for alloc in nc.m.functions[0].allocations:
        if not isinstance(alloc, mybir.MemoryLocationSet):
            continue
        name = alloc.memorylocations[0].name
        if alloc.kind == "ExternalInput":
            in_names.append(name)
        elif alloc.kind == "ExternalOutput":
            out_names.append(name)
            out_avals.append(
                jax.core.ShapedArray(
                    tuple(alloc.tensor_shape), mybir.dt.np(alloc.dtype)
                )
            )
    all_in = in_names + out_names

    def _body(*args):
        outs = bass2jax._bass_exec_p.bind(
            *args,
            out_avals=tuple(out_avals),
            in_names=tuple(all_in),
            out_names=tuple(out_names),
            lowering_input_output_aliases=(),
            sim_require_finite=True,
            sim_require_nnan=True,
            nc=nc,
        )
        return tuple(outs)

    devices = jax.devices()[:NCORES]
    mesh = Mesh(np.asarray(devices), ("core",))
    n_args = len(all_in)
    sharded = jax.jit(
        shard_map(
            _body,
            mesh=mesh,
            in_specs=(PartitionSpec("core"),) * n_args,
            out_specs=(PartitionSpec("core"),) * len(out_names),
            check_rep=False,
        ),
        keep_unused=True,
    )
    return sharded, mesh, in_names, out_names, out_avals


def bench(positions, colors, sizes, iters=50):
    """Steady-state per-execution wall time (s) over 8 cores + output."""
    import time as _time
    import jax
    from jax.sharding import NamedSharding, PartitionSpec

    positions = np.ascontiguousarray(np.asarray(positions, dtype=np.float32))
    colors = np.ascontiguousarray(np.asarray(colors, dtype=np.float32))
    sizes = np.ascontiguousarray(np.asarray(sizes, dtype=np.float32))
    nc = _get_bass()
    sharded, mesh, in_names, out_names, out_avals = _exec_fn(nc)

    lhs = _pack_lhs(positions, sizes).reshape(NCORES, BPC, 96, 128)
    col = _pack_colors(colors).reshape(NCORES, BPC, KC, NCHUNK, CH)
    feed = {
        "lhs": np.ascontiguousarray(lhs.transpose(0, 2, 1, 3)).reshape(
            NCORES * 96, BPC, 128
        ),
        "colors_pk": np.ascontiguousarray(
            col.transpose(0, 2, 1, 3, 4)
        ).reshape(NCORES * KC, BPC, NCHUNK, CH),
        "consts": np.concatenate([_np_grid96()] * NCORES, axis=0),
    }
    args = [feed[n] for n in in_names]
    args += [
        np.zeros((NCORES * a.shape[0], *a.shape[1:]), a.dtype) for a in out_avals
    ]
    sh = NamedSharding(mesh, PartitionSpec("core"))
    dargs = [jax.device_put(a, sh) for a in args]

    out = sharded(*dargs)
    jax.block_until_ready(out)
    img0 = _postprocess(
        np.asarray(out[0]).reshape(NCORES, BPC, H, W, CH)
    )

    times = []
    for _ in range(3):
        t0 = _time.perf_counter()
        for _ in range(iters):
            out = sharded(*dargs)
        jax.block_until_ready(out)
        times.append((_time.perf_counter() - t0) / iters)
    return min(times), img0
